# revision 12
# baseline (speedup 1.0000x reference)
"""Causal self-attention (B=2, T=2048, C=1024, H=16) on 8 TRN2 NeuronCores.

Sharding: 2 batches x 4 head-groups (4 heads each). Each core computes
qkv projection for its batch/head-slice, causal attention, and a partial
c_proj contribution; the host sums partials and adds b_proj.

Device layout (per core):
  xT  [C=1024, T=2048]  x[b] transposed (host-side), bf16
  wT  [C=1024, 768]     w_attn rows for this head slice, transposed: cols
                        [0:256]=Q feats, [256:512]=K, [512:768]=V, bf16
  bqkv [768]            matching bias slice (f32)
  wpT [256, 1024]       w_proj columns for this head slice, transposed, bf16
  out yT [1024, 2048]   partial (c_proj output)^T in bf16, summed over the
                        4 cores of the batch on host in f32

All matmuls run in bf16 (PE stays at full clock; fp32r-only stretches
throttle it to 1.2 GHz). PSUM accumulation is fp32; rel err ~6e-3 vs
the 2e-2 gate.
Causal structure: scores computed in S^T [key, query] orientation per
128-key-block x 512-query-chunk rectangles, only blocks intersecting the
causal triangle. Diagonal blocks are trimmed to the causal column range
[g:512]; the staircase mask-mul only covers the 128-col boundary block.
Softmax denominators come free from a ones-column appended to V (row 64
of the AV psum accumulator); normalization reads PSUM directly and
broadcasts the reciprocal row via an SBUF->SBUF DMA.
"""
import sys
import contextlib

sys.path.insert(0, "/opt/trn_rl_repo")

import numpy as np
import ml_dtypes

import concourse.bass as bass
import concourse.mybir as mybir
import concourse.tile as tile
from concourse import bacc
from concourse.bass_utils import run_bass_kernel_spmd

B, T, C, H = 2, 2048, 1024, 16
HD = 64
N_CORES = 8
HPC = 4          # heads per core
FPC = HPC * HD   # features per core = 256
QCH = 512        # query chunk
NQC = T // QCH   # 4
NKB = T // 128   # 16 k blocks
NCC = C // 128   # 8 contraction chunks
NTC = T // 512   # 4 token chunks

F32 = mybir.dt.float32
BF16 = mybir.dt.bfloat16

_CACHE: dict = {}


def _build():
    if "nc" in _CACHE:
        return _CACHE["nc"]
    nc = bacc.Bacc(None, target_bir_lowering=False, debug=False)

    xT_d = nc.dram_tensor("xT", [C, T], BF16, kind="ExternalInput").ap()
    wT_d = nc.dram_tensor("wT", [C, 3 * FPC], BF16, kind="ExternalInput").ap()
    bqkv_d = nc.dram_tensor("bqkv", [3 * FPC], F32, kind="ExternalInput").ap()
    wpT_d = nc.dram_tensor("wpT", [FPC, C], BF16, kind="ExternalInput").ap()
    yT_d = nc.dram_tensor("yT", [C, T], BF16, kind="ExternalOutput").ap()

    # staircase masks for the 4 diagonal-block offsets g = 0,128,256,384:
    # mask[g//128][i, j] = 1 if i <= j - g else 0  ([key, query] layout).
    # Only the 128-col boundary block [g:g+128] is ever partial, so store
    # just that block per g.
    i_idx = np.arange(128)[:, None]
    j_idx = np.arange(128)[None, :]
    masks_np = np.stack(
        [(i_idx <= j_idx).astype(np.float32) for _ in range(1)]
    ).astype(ml_dtypes.bfloat16)  # single triangular block, same for all g
    masks_d = nc.inline_tensor(masks_np, name="masks").ap()

    with tile.TileContext(nc) as tc:
        with contextlib.ExitStack() as ctx:
            consts = ctx.enter_context(tc.tile_pool(name="consts", bufs=1))
            xpool = ctx.enter_context(tc.tile_pool(name="x", bufs=1))
            qkpool = ctx.enter_context(tc.tile_pool(name="qk", bufs=1))
            vpool = ctx.enter_context(tc.tile_pool(name="v", bufs=1))
            ppool = ctx.enter_context(tc.tile_pool(name="p", bufs=2))
            ypool = ctx.enter_context(tc.tile_pool(name="y", bufs=2))
            opool = ctx.enter_context(tc.tile_pool(name="o", bufs=3))
            rpool = ctx.enter_context(tc.tile_pool(name="r", bufs=2))
            big_ps = ctx.enter_context(tc.tile_pool(name="big_ps", bufs=2, space="PSUM"))
            s_ps = ctx.enter_context(tc.tile_pool(name="s_ps", bufs=2, space="PSUM"))
            y_ps = ctx.enter_context(tc.tile_pool(name="y_ps", bufs=2, space="PSUM"))

            # ---- DMA helpers: alternate between the two hardware DGE
            # queues (SP + Activation) so transfers run in parallel ----
            dma_state = {"i": 0}

            def dma(out, in_):
                # alternate queues; use only while ACT has no exp work
                eng = nc.sync if dma_state["i"] % 2 == 0 else nc.scalar
                dma_state["i"] += 1
                eng.dma_start(out=out, in_=in_)

            def dma_s(out, in_):
                nc.sync.dma_start(out=out, in_=in_)

            # ---- DMA order: what the first matmuls need comes first ----
            # wt is split into 128-col chunks so the fb=0 QKV group can
            # start after ~1.3MB instead of the full 2.6MB weight+x load.
            wt = {}
            xt = {}
            for cc in range(NCC):
                wt[cc] = consts.tile([128, 3 * FPC], BF16, tag=f"w{cc}", name=f"w{cc}")
                dma(wt[cc][:, 0:128], wT_d[cc * 128:(cc + 1) * 128, 0:128])
                xt[(cc, 0)] = xpool.tile([128, 512], BF16, tag=f"x{cc}_0", name=f"x{cc}_0")
                dma(xt[(cc, 0)][:], xT_d[cc * 128:(cc + 1) * 128, 0:512])
            qk_bias = {}
            for fb in range(4):
                qk_bias[fb] = consts.tile([128, 1], F32, tag=f"qkb{fb}", name=f"qkb{fb}")
                dma(qk_bias[fb][:], bqkv_d[fb * 128:(fb + 1) * 128].unsqueeze(-1))
            for fb in range(1, 4):
                for cc in range(NCC):
                    dma(wt[cc][:, fb * 128:(fb + 1) * 128],
                        wT_d[cc * 128:(cc + 1) * 128, fb * 128:(fb + 1) * 128])
            for cc in range(NCC):
                dma(wt[cc][:, 512:768], wT_d[cc * 128:(cc + 1) * 128, 512:768])
            v_bias = consts.tile([128, FPC], F32, tag="vbias")
            dma(v_bias[:], bqkv_d[512:768].partition_broadcast(128))
            for cc in range(NCC):
                xt[(cc, 1)] = xpool.tile([128, 512], BF16, tag=f"x{cc}_1", name=f"x{cc}_1")
                dma(xt[(cc, 1)][:], xT_d[cc * 128:(cc + 1) * 128, 512:1024])
            masks = consts.tile([128, 128], BF16, tag="masks")
            dma_s(masks[:], masks_d[0])
            wp = {}
            for hc in range(2):
                wp[hc] = consts.tile([128, C], BF16, tag=f"wp{hc}", name=f"wp{hc}")
                dma_s(wp[hc][:], wpT_d[hc * 128:(hc + 1) * 128, :])
            for tc_i in range(2, NTC):
                for cc in range(NCC):
                    xt[(cc, tc_i)] = xpool.tile([128, 512], BF16, tag=f"x{cc}_{tc_i}", name=f"x{cc}_{tc_i}")
                    dma_s(xt[(cc, tc_i)][:], xT_d[cc * 128:(cc + 1) * 128, tc_i * 512:(tc_i + 1) * 512])

            # ---- QKV projection ----
            # feature-major Q^T, K^T: qk[(fb, tc)] [128, 512], fb 0..1 = Q
            # (heads 0-1, 2-3), fb 2..3 = K
            qk = {}
            vt_by_tb = {}

            def emit_qkv(tc_i):
                for fb in range(4):
                    ps = big_ps.tile([128, 512], F32, tag="bigps")
                    for cc in range(NCC):
                        nc.tensor.matmul(
                            ps[:],
                            wt[cc][:, fb * 128:(fb + 1) * 128],
                            xt[(cc, tc_i)][:],
                            start=(cc == 0),
                            stop=(cc == NCC - 1),
                        )
                    qk[(fb, tc_i)] = qkpool.tile([128, 512], BF16, tag=f"qk{fb}_{tc_i}", name=f"qk{fb}_{tc_i}")
                    nc.vector.tensor_scalar_add(qk[(fb, tc_i)][:], ps[:], qk_bias[fb][:])
                # token-major V_ext tiles [128 tokens, 4 heads, 66] (64 V cols,
                # col 64 = ones for the softmax denominator, col 65 pad)
                for tb in range(tc_i * 4, tc_i * 4 + 4):
                    ps = big_ps.tile([128, FPC], F32, tag="bigps")
                    for cc in range(NCC):
                        nc.tensor.matmul(
                            ps[:],
                            xt[(cc, tc_i)][:, (tb % 4) * 128:(tb % 4 + 1) * 128],
                            wt[cc][:, 512:768],
                            start=(cc == 0),
                            stop=(cc == NCC - 1),
                        )
                    vt = vpool.tile([128, HPC, 66], BF16, tag=f"v{tb}")
                    nc.vector.tensor_add(
                        vt[:, :, 0:64],
                        ps[:].rearrange("p (h d) -> p h d", h=HPC),
                        v_bias[:].rearrange("p (h d) -> p h d", h=HPC),
                    )
                    nc.vector.memset(vt[:, :, 64:65], 1.0)
                    vt_by_tb[tb] = vt


            yT_by_qc = {}

            def emit_attn(qc):
                yT_pair = {}
                for hc in range(2):
                    yT_pair[hc] = ypool.tile([128, 512], BF16, tag=f"yp{hc}", name=f"yp{hc}_{qc}")
                yT_by_qc[qc] = yT_pair
                kmax = 4 * (qc + 1)
                for hp in range(2):
                    ys = {hb: y_ps.tile([65, 512], F32, tag="yps", name=f"yps{qc}_{hp}_{hb}") for hb in range(2)}

                    def emit_av(kb, pt, gc):
                        for hb in range(2):
                            nc.tensor.matmul(
                                ys[hb][:, gc:],
                                vt_by_tb[kb][:, 2 * hp + hb, 0:65],
                                pt[:, hb * 512 + gc:(hb + 1) * 512],
                                start=(kb == 0),
                                stop=(kb == kmax - 1),
                                skip_group_check=True,
                            )

                    pend = None  # AV is delayed one k-block so exp latency hides
                    for kb in range(kmax):
                        g = kb * 128 - qc * 512  # diag offset; >0: cols [0:g] fully masked
                        gc = max(g, 0)
                        sp = s_ps.tile([128, 1024], F32, tag="sps")
                        for hb in range(2):
                            rows = slice(hb * 64, hb * 64 + 64)
                            nc.tensor.matmul(
                                sp[:, hb * 512 + gc:(hb + 1) * 512],
                                qk[(2 + hp, kb // 4)][rows, (kb % 4) * 128:(kb % 4 + 1) * 128],
                                qk[(hp, qc)][rows, gc:],
                                start=True,
                                stop=True,
                            )
                        pt = ppool.tile([128, 1024], BF16, tag=f"p{kb % 6}", name=f"p{qc}_{hp}_{kb}")
                        if gc == 0:
                            nc.scalar.activation(
                                pt[:], sp[:], mybir.ActivationFunctionType.Exp,
                                scale=0.125,
                            )
                        else:
                            for hb in range(2):
                                base = hb * 512
                                nc.scalar.activation(
                                    pt[:, base + gc:base + 512],
                                    sp[:, base + gc:base + 512],
                                    mybir.ActivationFunctionType.Exp,
                                    scale=0.125,
                                )
                        if g > -128:
                            for hb in range(2):
                                base = hb * 512
                                # only the 128-col boundary block is partial
                                nc.vector.tensor_mul(
                                    pt[:, base + gc:base + gc + 128],
                                    pt[:, base + gc:base + gc + 128],
                                    masks[:],
                                )
                        if pend is not None:
                            emit_av(*pend)
                        pend = (kb, pt, gc)
                    emit_av(*pend)
                    for hb in range(2):
                        yp = ys[hb]
                        y_sb = rpool.tile([65, 512], F32, tag="y_sb")
                        nc.vector.tensor_copy(y_sb[:], yp[:])
                        denom0 = rpool.tile([1, 512], F32, tag="denom0")
                        nc.sync.dma_start(out=denom0[:], in_=y_sb[64:65, :])
                        recip = rpool.tile([1, 512], F32, tag="recip")
                        nc.vector.reciprocal_approx_fast(out=recip[:], in_=denom0[:])
                        recip_b = rpool.tile([64, 512], F32, tag="recip_b")
                        nc.gpsimd.partition_broadcast(recip_b[:], recip[:])
                        nc.vector.tensor_mul(
                            yT_pair[hp][hb * 64:(hb + 1) * 64, :], y_sb[0:64, :], recip_b[:]
                        )

            def emit_cproj_ob(qc, ob, tail=False):
                yT_pair = yT_by_qc[qc]
                op = big_ps.tile([128, 512], F32, tag="bigps")
                for hc in range(2):
                    nc.tensor.matmul(
                        op[:],
                        wp[hc][:, ob * 128:(ob + 1) * 128],
                        yT_pair[hc][:],
                        start=(hc == 0),
                        stop=(hc == 1),
                    )
                ot = opool.tile([128, 512], BF16, tag="ot")
                if tail and ob % 2 == 1:
                    nc.scalar.copy(ot[:], op[:])
                else:
                    nc.vector.tensor_copy(ot[:], op[:])
                d = dma if tail else dma_s
                d(yT_d[ob * 128:(ob + 1) * 128, qc * 512:(qc + 1) * 512], ot[:])


            # software-pipelined order: each attention chunk is emitted
            # after the qkv chunk that follows it, so exp (ACT) overlaps
            # qkv matmuls (PE). c_proj(qc) is deferred one segment so the
            # normalization chain never stalls the in-order PE queue.
            # attn(0) is the smallest chunk; it goes last, and cp3/cp0
            # fill the tail.
            emit_qkv(0)
            emit_qkv(1)
            emit_attn(1)
            emit_qkv(2)
            for ob in range(8):
                emit_cproj_ob(1, ob)
            emit_attn(2)
            emit_qkv(3)
            for ob in range(8):
                emit_cproj_ob(2, ob)
            emit_attn(3)
            emit_attn(0)
            # tail: interleave cp3 (ready) with cp0 (waiting on attn0's
            # normalization) so the in-order PE queue always has work
            for ob in range(8):
                emit_cproj_ob(3, ob, tail=True)
                emit_cproj_ob(0, ob, tail=True)
    nc.compile()
    _CACHE["nc"] = nc
    return nc


def _make_in_maps(x, w_attn, b_attn, w_proj):
    bf = ml_dtypes.bfloat16
    in_maps = []
    for core in range(N_CORES):
        b, s = core // 4, core % 4
        f0 = FPC * s
        xT = np.ascontiguousarray(x[b].T.astype(bf))
        wT = np.ascontiguousarray(
            np.concatenate(
                [
                    w_attn[f0:f0 + FPC],
                    w_attn[C + f0:C + f0 + FPC],
                    w_attn[2 * C + f0:2 * C + f0 + FPC],
                ],
                axis=0,
            ).T.astype(bf)
        )
        bqkv = np.ascontiguousarray(
            np.concatenate(
                [
                    b_attn[f0:f0 + FPC],
                    b_attn[C + f0:C + f0 + FPC],
                    b_attn[2 * C + f0:2 * C + f0 + FPC],
                ]
            )
        )
        wpT = np.ascontiguousarray(w_proj[:, f0:f0 + FPC].T.astype(bf))
        in_maps.append({"xT": xT, "wT": wT, "bqkv": bqkv, "wpT": wpT})
    return in_maps


def kernel(x, w_attn, b_attn, w_proj, b_proj):
    nc = _build()
    in_maps = _make_in_maps(x, w_attn, b_attn, w_proj)
    _CACHE["in_maps"] = in_maps

    res = run_bass_kernel_spmd(nc, in_maps, list(range(N_CORES)))
    out = np.empty((B, T, C), dtype=np.float32)
    for b in range(B):
        acc = res.results[4 * b]["yT"].astype(np.float32)
        for s in range(1, 4):
            acc = acc + res.results[4 * b + s]["yT"].astype(np.float32)
        out[b] = acc.T + b_proj[None, :]
    return out
